# revision 1
# baseline (speedup 1.0000x reference)
"""Differentiable envelope follower on 8 Trainium2 NeuronCores.

Algorithm: the per-sample recurrence
    env[t] = c[t]*env[t-1] + (1-c[t])*|x[t]|,  c[t] = ca if |x[t]| > env[t-1] else cr
is solved by policy iteration: derive attack/release decisions elementwise
from a trajectory guess, solve the resulting LINEAR recurrence exactly with
tensor_tensor_scan (a DVE-only instruction), repeat.  A causal EMA proxy
(gamma-scaled running mean of |x|) provides the initial guess, which makes
TWO solves sufficient (max rel err ~3e-4 vs the 2e-2 gate, validated in fp16
at full scale and on an independent random draw).

Per solve, each elementwise pass is a single legal instruction:
    c  = cr + dc*dec      (Act activation, Copy with scale/bias)
    kk = 1-c = kr+|dc|dec (tensor_scalar, two scalars)
    d1 = kk*lt            (tensor_tensor mult; lt = |x| f16 plane)
Solve-0 decisions are an all-f16 2x-mode is_gt on DVE against the proxy.
Solve-1 decisions are stored as sign(|x[t]|-env[t-1]) in {-1,0,+1}: Pool does
the subtract, Act the Sign (comparison ops are DVE-only, so this keeps the
second decision pass off the DVE whose three scans are the critical budget).
Sign ties (diff==0) are exactly neutral since c+kk=1.

Engine budget per core (cost model): DVE = EMA scan + solve scans + one 2x
compare (~115us); Act = abs + 2 c-passes + Sign (~118us); Pool = products +
diffs + kk (~127us).  y is stored f16 (upcast on host) to halve store traffic
on the shared DMA bus; x loads stream on the SP queue.

Sharding: L=480000 split across 8 cores (60000 each); 64 batch rows x 2
L-halves fill the 128 SBUF partitions ([128, 30000] per core).  Chunk-boundary
states are exchanged between solves with a tiny AllGather (512B).
"""

import math
import numpy as np

# ---- problem constants (hardcoded per contract) ----
B = 64
L = 480000
NCORES = 8
KCORE = L // NCORES          # 60000 per core
HALF = KCORE // 2            # 30000 per partition-row
P = 128

# ---- tunables ----
YS = 1024.0                  # y stored as YS*env in f16 (avoids subnormals)
TF = 1000                    # free-dim tile size (must divide HALF)
ITERS = 2                    # number of linear solves (policy iterations)
EQ = 1.42                    # equilibrium level of the init-guess ramp
TAU = 5600.0                 # time constant of the init-guess ramp
CM_SAMPLES = 3000.0          # EMA proxy time constant (samples)
GAMMA = 1.8                  # proxy -> envelope threshold scale
DEC_DVE = 3                  # solve-1 dec on DVE is_gt every j%DEC_DVE==0 (else Pool diff + Act Sign)

_RUN_KWARGS = {}             # test.py can set {"trace": True}
_cache = {}


def _coeffs(raw_attack, raw_release, sample_rate):
    # Mirror reference._coefficients exactly (same jax ops, on CPU).
    import jax
    import jax.numpy as jnp

    with jax.default_device(jax.devices("cpu")[0]):
        attack_ms = 0.1 + jax.nn.sigmoid(jnp.asarray(np.float32(raw_attack))) * 49.9
        release_ms = 10.0 + jax.nn.sigmoid(jnp.asarray(np.float32(raw_release))) * 490.0
        attack_samples = attack_ms * float(sample_rate) / 1000.0
        release_samples = release_ms * float(sample_rate) / 1000.0
        ca = jnp.exp(-1.0 / attack_samples)
        cr = jnp.exp(-1.0 / release_samples)
        return float(ca), float(cr)


def _build(ca, cr):
    import concourse.bass as bass
    import concourse.bacc as bacc
    import concourse.tile as tile
    from concourse import mybir
    from concourse.tile_rust import add_dep_helper
    from contextlib import ExitStack

    f32 = mybir.dt.float32
    f16 = mybir.dt.float16
    Alu = mybir.AluOpType
    Act = mybir.ActivationFunctionType
    NT = HALF // TF
    assert NT * TF == HALF

    dc = float(np.float32(ca) - np.float32(cr))          # < 0
    adc = -dc                                            # |dc| = cr - ca
    kr = float(np.float32(1.0) - np.float32(cr))
    cm = float(np.float32(math.exp(-1.0 / CM_SAMPLES)))
    YSf = np.float32(YS)
    gcm = float(np.float32(GAMMA) * (np.float32(1.0) - np.float32(cm)))
    # sign-convention (s in {-1,0,1}) affine constants for solve 1
    c1_bias = float(np.float32(cr) + np.float32(dc) / 2)
    c1_scale = float(np.float32(dc) / 2)
    k1_bias = float(YS * (np.float32(kr) + np.float32(adc) / 2))
    k1_scale = float(YS * (np.float32(adc) / 2))

    nc = bacc.Bacc("TRN2", target_bir_lowering=False, debug=False,
                   num_devices=NCORES)

    x_in = nc.dram_tensor("xc", [P, HALF], f32, kind="ExternalInput")
    seed_in = nc.dram_tensor("seed0", [P, 1], f32, kind="ExternalInput")
    selw_in = nc.dram_tensor("selw", [P, NCORES], f32, kind="ExternalInput")
    y_out = nc.dram_tensor("yc", [P, HALF], f16, kind="ExternalOutput")
    bnd_loc = nc.dram_tensor("bnd_loc", [P], f32)
    bnd_all = nc.dram_tensor("bnd_all", [NCORES, P], f32, addr_space="Shared")

    nc.alloc_semaphore("bnd_dma")
    nc.alloc_semaphore("bnd_cc")
    groups = [list(range(NCORES))]

    with tile.TileContext(nc) as tc:
        with ExitStack() as ctx:
            pers = ctx.enter_context(tc.tile_pool(name="pers", bufs=1))
            xp = ctx.enter_context(tc.tile_pool(name="x", bufs=2))
            pdp = ctx.enter_context(tc.tile_pool(name="pd", bufs=2))
            pgp = ctx.enter_context(tc.tile_pool(name="pg", bufs=2))
            dbp = ctx.enter_context(tc.tile_pool(name="db", bufs=2))
            cp = ctx.enter_context(tc.tile_pool(name="c", bufs=3))
            kkp = ctx.enter_context(tc.tile_pool(name="kk", bufs=3))
            d1p = ctx.enter_context(tc.tile_pool(name="d1", bufs=3))
            efp = ctx.enter_context(tc.tile_pool(name="ef", bufs=3))
            dfp = ctx.enter_context(tc.tile_pool(name="df", bufs=2))
            yhp = ctx.enter_context(tc.tile_pool(name="yh", bufs=2))
            bcolp = ctx.enter_context(tc.tile_pool(name="bcol", bufs=2))
            seedp = ctx.enter_context(tc.tile_pool(name="seed", bufs=2))

            ltp = pers.tile([P, HALF], f16, tag="lt")     # |x|
            decp = pers.tile([P, HALF], f16, tag="dec")   # sign(|x|-env) plane
            cmt = pers.tile([P, TF], f32, tag="cmt")      # EMA scan multiplier
            selw_sb = pers.tile([P, NCORES], f32, tag="selw")
            bnd_sb = pers.tile([P, NCORES], f32, tag="bnd")
            sel_t = pers.tile([P, NCORES], f32, tag="sel")

            seed0_t = seedp.tile([P, 1], f32, tag="s0")
            nc.gpsimd.dma_start(seed0_t[:, :], seed_in[:, :])
            nc.gpsimd.dma_start(selw_sb[:, :], selw_in[:, :])
            nc.gpsimd.memset(cmt[:, :], cm)

            # ---------- solve 0 + proxy ----------
            # emission is software-pipelined one step: tile j's scan/decision
            # ops are emitted while tile j+1's prep runs, so no engine queue
            # blocks at an unready instruction.
            prep = {}

            def emit_prep(j):
                a = j * TF
                lts = ltp[:, a:a + TF]
                x_t = xp.tile([P, TF], f32, tag="x")
                nc.sync.dma_start(x_t[:, :], x_in[:, a:a + TF])
                nc.scalar.activation(lts, x_t[:, :], Act.Abs)
                pd_t = pdp.tile([P, TF], f16, tag="pd")
                nc.gpsimd.tensor_scalar(pd_t[:, :], lts, gcm, None,
                                        op0=Alu.mult)
                pg_t = pgp.tile([P, TF], f16, tag="pg")
                pg_init = (prep[j - 1]["pg"][:, TF - 1:TF] if j > 0
                           else seed0_t[:, 0:1])
                nc.vector.tensor_tensor_scan(pg_t[:, :], cmt[:, :],
                                             pd_t[:, :], pg_init,
                                             op0=Alu.mult, op1=Alu.add)
                db_t = dbp.tile([P, TF], f16, tag="db")
                nc.vector.tensor_tensor(db_t[:, 1:], lts[:, 1:],
                                        pg_t[:, :TF - 1], op=Alu.is_gt)
                prev_col = (prep[j - 1]["pg"][:, TF - 1:TF] if j > 0
                            else seed0_t[:, 0:1])
                nc.vector.tensor_tensor(db_t[:, 0:1], lts[:, 0:1], prev_col,
                                        op=Alu.is_gt)
                c_t = cp.tile([P, TF], f32, tag="c")
                nc.scalar.activation(c_t[:, :], db_t[:, :], Act.Copy,
                                     bias=float(cr), scale=dc)
                kk_t = kkp.tile([P, TF], f32, tag="kk")
                nc.gpsimd.tensor_scalar(kk_t[:, :], db_t[:, :], adc,
                                        float(kr), op0=Alu.mult, op1=Alu.add)
                d1_t = d1p.tile([P, TF], f32, tag="d1")
                nc.gpsimd.tensor_tensor(d1_t[:, :], kk_t[:, :], lts,
                                        op=Alu.mult)
                prep[j] = {"pg": pg_t, "c": c_t, "d1": d1_t}
                prep.pop(j - 2, None)

            envs = {}

            def emit_scan(j):
                a = j * TF
                lts = ltp[:, a:a + TF]
                env_t = efp.tile([P, TF], f32, tag="ef")
                init_ap = (envs[j - 1][:, TF - 1:TF] if j > 0
                           else seed0_t[:, 0:1])
                nc.vector.tensor_tensor_scan(env_t[:, :], prep[j]["c"][:, :],
                                             prep[j]["d1"][:, :], init_ap,
                                             op0=Alu.mult, op1=Alu.add)
                # solve-1 decisions (shifted): DVE is_gt on some tiles
                # ({0,1} convention), Pool diff + Act Sign on the rest
                # ({-1,0,1} convention); solve-1 constants match per tile.
                if j % DEC_DVE == 0:
                    nc.vector.tensor_tensor(decp[:, a + 1:a + TF], lts[:, 1:],
                                            env_t[:, :TF - 1], op=Alu.is_gt)
                    pcol = (envs[j - 1][:, TF - 1:TF] if j > 0
                            else seed0_t[:, 0:1])
                    nc.vector.tensor_tensor(decp[:, a:a + 1], lts[:, 0:1],
                                            pcol, op=Alu.is_gt)
                else:
                    df_t = dfp.tile([P, TF], f32, tag="df")
                    nc.gpsimd.tensor_tensor(df_t[:, 1:], lts[:, 1:],
                                            env_t[:, :TF - 1],
                                            op=Alu.subtract)
                    nc.gpsimd.tensor_tensor(df_t[:, 0:1], lts[:, 0:1],
                                            envs[j - 1][:, TF - 1:TF],
                                            op=Alu.subtract)
                    nc.scalar.activation(decp[:, a:a + TF], df_t[:, :],
                                         Act.Sign)
                envs[j] = env_t
                envs.pop(j - 2, None)

            emit_prep(0)
            for j in range(1, NT):
                emit_prep(j)
                emit_scan(j - 1)
            emit_scan(NT - 1)
            env_last = envs[NT - 1]

            # ---------- solve-1 prep (emitted before the exchange so all
            # engines stay busy during the 15us collective) ----------
            prep1 = {}

            def emit_prep1(j):
                a = j * TF
                lts = ltp[:, a:a + TF]
                dsl = decp[:, a:a + TF]
                if j % DEC_DVE == 0:    # {0,1} decisions
                    cb, cs = float(cr), dc
                    kb, ks = float(YS * np.float32(kr)), float(YS * np.float32(adc))
                else:                    # sign {-1,0,1} decisions
                    cb, cs = c1_bias, c1_scale
                    kb, ks = k1_bias, k1_scale
                c_t = cp.tile([P, TF], f32, tag="c")
                nc.scalar.activation(c_t[:, :], dsl, Act.Copy, bias=cb,
                                     scale=cs)
                kk_t = kkp.tile([P, TF], f32, tag="kk")
                nc.vector.tensor_scalar(kk_t[:, :], dsl, ks, kb,
                                        op0=Alu.mult, op1=Alu.add)
                d1_t = d1p.tile([P, TF], f32, tag="d1")
                d1_eng = nc.vector if 2 <= j <= 5 else nc.gpsimd
                d1_eng.tensor_tensor(d1_t[:, :], kk_t[:, :], lts,
                                     op=Alu.mult)
                prep1[j] = {"c": c_t, "d1": d1_t}

            LOOK = 2
            for j in range(min(LOOK, NT)):
                emit_prep1(j)
            # ---------- boundary exchange ----------
            bcol = bcolp.tile([P, 1], f32, tag="bcol")
            nc.vector.tensor_copy(bcol[:, :], env_last[:, TF - 1:TF])
            st1 = nc.sync.dma_start(bnd_loc[0:64], bcol[64:128, 0:1])
            st2 = nc.sync.dma_start(bnd_loc[64:128], bcol[0:64, 0:1])
            cc = nc.gpsimd.collective_compute(
                "AllGather", mybir.AluOpType.bypass,
                replica_groups=groups,
                ins=[bnd_loc[:]], outs=[bnd_all[:, :]],
            )
            add_dep_helper(cc.ins, st1.ins, sync=True,
                           reason="collective after bnd stores")
            add_dep_helper(cc.ins, st2.ins, sync=True,
                           reason="collective after bnd stores")
            for g in range(NCORES):
                ld = nc.sync.dma_start(bnd_sb[:, g:g + 1], bnd_all[g, :])
                add_dep_helper(ld.ins, cc.ins, sync=True,
                               reason="bnd load after collective")
            nc.vector.tensor_tensor(sel_t[:, :], bnd_sb[:, :], selw_sb[:, :],
                                    op=Alu.mult)
            seed_t = seedp.tile([P, 1], f32, tag="sx")
            nc.vector.tensor_reduce(seed_t[:, :], sel_t[:, :],
                                    axis=mybir.AxisListType.X, op=Alu.add)

            # ---------- solve 1 (final) ----------
            seedy_t = seedp.tile([P, 1], f32, tag="sy")
            nc.vector.tensor_scalar(seedy_t[:, :], seed_t[:, :], float(YSf),
                                    None, op0=Alu.mult)
            prev_y = None
            for j in range(NT):
                a = j * TF
                y_t = yhp.tile([P, TF], f16, tag="yh")
                init_ap = (prev_y[:, TF - 1:TF] if j > 0 else seedy_t[:, 0:1])
                nc.vector.tensor_tensor_scan(y_t[:, :], prep1[j]["c"][:, :],
                                             prep1[j]["d1"][:, :], init_ap,
                                             op0=Alu.mult, op1=Alu.add)
                nc.gpsimd.dma_start(y_out[:, a:a + TF], y_t[:, :])
                prev_y = y_t
                prep1.pop(j, None)
                if j + LOOK < NT:
                    emit_prep1(j + LOOK)
    nc.finalize()
    return nc


def _in_maps(x, ca, cr):
    x = np.ascontiguousarray(np.asarray(x, dtype=np.float32))
    maps = []
    t0 = np.empty(P, np.float64)
    for c in range(NCORES):
        t0[:64] = c * KCORE
        t0[64:] = c * KCORE + HALF
        seed0 = (EQ * (1.0 - np.exp(-t0 / TAU))).astype(np.float32)[:, None]
        selw = np.zeros((P, NCORES), np.float32)
        if c > 0:
            selw[:64, c - 1] = 1.0
        selw[64:, c] = 1.0
        s = c * KCORE
        xc = np.concatenate([x[:, s:s + HALF], x[:, s + HALF:s + KCORE]],
                            axis=0)
        maps.append({
            "xc": np.ascontiguousarray(xc),
            "seed0": seed0,
            "selw": selw,
        })
    return maps


def kernel(x, raw_attack, raw_release, sample_rate):
    from concourse.bass_utils import run_bass_kernel_spmd

    ca, cr = _coeffs(raw_attack, raw_release, sample_rate)
    key = (round(ca, 12), round(cr, 12), TF, ITERS, GAMMA, CM_SAMPLES)
    if key not in _cache:
        _cache[key] = _build(ca, cr)
    nc = _cache[key]

    maps = _in_maps(x, ca, cr)
    res = run_bass_kernel_spmd(nc, maps, list(range(NCORES)), **_RUN_KWARGS)
    kernel.last_results = res

    y = np.empty((B, L), np.float32)
    for c in range(NCORES):
        yc = np.asarray(res.results[c]["yc"], dtype=np.float32) * np.float32(1.0 / YS)
        s = c * KCORE
        y[:, s:s + HALF] = yc[:64]
        y[:, s + HALF:s + KCORE] = yc[64:]
    return y



# revision 10
# speedup vs baseline: 1.4456x; 1.4456x over previous
"""Differentiable envelope follower on 8 Trainium2 NeuronCores.

Algorithm: the per-sample recurrence
    env[t] = c[t]*env[t-1] + (1-c[t])*|x[t]|,  c[t] = ca if |x[t]| > env[t-1] else cr
is solved by policy iteration: derive attack/release decisions elementwise
from a trajectory guess, solve the resulting LINEAR recurrence exactly with
tensor_tensor_scan (DVE-only), repeat.  All decisions are THRESHOLD compares
(tensor_scalar is_gt with a [P,1] ptr -- legal on Pool AND 4x-fast on DVE):
  solve 0: thr = decimated-EMA proxy of |x|, held per 1000-sample half-tile
  solve 1: thr = env0 itself, held per 250-sample block (plus an exact
           per-element compare for the first tile, where the envelope rises
           from zero and relative errors are magnified)
Max rel err 4.4e-3 vs the 2e-2 gate (validated in numpy at full scale).

Cost structure (vs 180.6us baseline):
  - |x| and the f32->f16 downcast happen on the HOST; lt = f16(16*|x|).
  - both scans run in y' = env/adc units (adc = cr-ca) so the kk-affine
    collapses into the decision op:  t' = (lt > thr) + kr/adc  is ONE 4x
    tensor_scalar, d' = t'*lt one tensor_tensor, c = 1 - adc*t' one Act
    Copy.  Host multiplies the output by adc/16.
  - scans must run on DVE (ISA); d-products run on Pool, c-affines on Act,
    decision-TS ops split DVE/Pool -- all four engines near-balanced.

Sharding: L=480000 split across 8 cores (60000 each); 64 batch rows x 2
L-halves fill the 128 SBUF partitions ([128, 30000] per core).  Chunk-
boundary states are exchanged between solves with a tiny AllGather.
"""

import math
import numpy as np

# ---- problem constants (hardcoded per contract) ----
B = 64
L = 480000
NCORES = 8
KCORE = L // NCORES          # 60000 per core
HALF = KCORE // 2            # 30000 per partition-row
P = 128

# ---- tunables ----
ALPHA = 16.0                 # lt = ALPHA*|x| stored f16
TF = 2000                    # free-dim tile size (must divide HALF)
THALF = TF // 2              # solve-0 threshold hold (samples)
DEC = 8                      # proxy decimation stride
TFD = TF // DEC              # decimated proxy tile length (250)
BL1 = 250                    # solve-1 threshold hold (samples)
NB1 = TF // BL1              # solve-1 blocks per tile (8)
CM_SAMPLES = 4500.0          # EMA proxy time constant (samples)
GAMMA = 1.8                  # proxy -> envelope threshold scale
EQ = 1.42                    # equilibrium level of the init-guess ramp
TAU = 5600.0                 # time constant of the init-guess ramp
LOOK = 2                     # solve-1 tiles emitted before the exchange
D11_DVE = 6                  # tiles whose d11 TT runs on DVE (rest Pool)

_RUN_KWARGS = {}             # test.py can set {"trace": True}
_cache = {}


def _coeffs(raw_attack, raw_release, sample_rate):
    # Mirror reference._coefficients exactly (same jax ops, on CPU).
    import jax
    import jax.numpy as jnp

    with jax.default_device(jax.devices("cpu")[0]):
        attack_ms = 0.1 + jax.nn.sigmoid(jnp.asarray(np.float32(raw_attack))) * 49.9
        release_ms = 10.0 + jax.nn.sigmoid(jnp.asarray(np.float32(raw_release))) * 490.0
        attack_samples = attack_ms * float(sample_rate) / 1000.0
        release_samples = release_ms * float(sample_rate) / 1000.0
        ca = jnp.exp(-1.0 / attack_samples)
        cr = jnp.exp(-1.0 / release_samples)
        return float(ca), float(cr)


def _build(ca, cr):
    import concourse.bass as bass
    import concourse.bacc as bacc
    import concourse.tile as tile
    from concourse import mybir
    from concourse.tile_rust import add_dep_helper
    from contextlib import ExitStack

    f32 = mybir.dt.float32
    f16 = mybir.dt.float16
    Alu = mybir.AluOpType
    Act = mybir.ActivationFunctionType
    NT = HALF // TF
    assert NT * TF == HALF

    adc = float(np.float32(cr) - np.float32(ca))         # > 0
    kr = float(np.float32(1.0) - np.float32(cr))
    bk = float(np.float32(kr) / np.float32(adc))         # t' offset (~0.109)
    cm = float(np.float32(math.exp(-DEC / CM_SAMPLES)))  # per decimated step
    gcm = float(np.float32(GAMMA) * (np.float32(1.0) - np.float32(cm)))

    nc = bacc.Bacc("TRN2", target_bir_lowering=False, debug=False,
                   num_devices=NCORES)

    x_in = nc.dram_tensor("xc", [P, HALF], f16, kind="ExternalInput")
    seed_in = nc.dram_tensor("seed0", [P, 1], f32, kind="ExternalInput")
    selw_in = nc.dram_tensor("selw", [P, NCORES], f32, kind="ExternalInput")
    y_out = nc.dram_tensor("yc", [P, HALF], f16, kind="ExternalOutput")
    bnd_loc = nc.dram_tensor("bnd_loc", [P], f32)
    bnd_all = nc.dram_tensor("bnd_all", [NCORES, P], f32, addr_space="Shared")

    nc.alloc_semaphore("bnd_dma")
    nc.alloc_semaphore("bnd_cc")
    groups = [list(range(NCORES))]

    with tile.TileContext(nc) as tc:
        with ExitStack() as ctx:
            pers = ctx.enter_context(tc.tile_pool(name="pers", bufs=1))
            pgp = ctx.enter_context(tc.tile_pool(name="pg", bufs=2))
            pdp = ctx.enter_context(tc.tile_pool(name="pd", bufs=2))
            ttp = ctx.enter_context(tc.tile_pool(name="tt", bufs=2))
            cp = ctx.enter_context(tc.tile_pool(name="c", bufs=2))
            dp = ctx.enter_context(tc.tile_pool(name="d1", bufs=3))
            efp = ctx.enter_context(tc.tile_pool(name="ef", bufs=2))
            yhp = ctx.enter_context(tc.tile_pool(name="yh", bufs=2))
            xtr = ctx.enter_context(tc.tile_pool(name="xtr", bufs=2))
            bcolp = ctx.enter_context(tc.tile_pool(name="bcol", bufs=2))
            seedp = ctx.enter_context(tc.tile_pool(name="seed", bufs=2))

            ltp = pers.tile([P, HALF], f16, tag="lt", name="ltp")
            thrp = pers.tile([P, NT * NB1], f32, tag="thr", name="thrp")
            cmt = pers.tile([P, TFD], f32, tag="cmt", name="cmt")
            selw_sb = pers.tile([P, NCORES], f32, tag="selw", name="selw_sb")
            bnd_sb = pers.tile([P, NCORES], f32, tag="bnd", name="bnd_sb")
            sel_t = pers.tile([P, NCORES], f32, tag="sel", name="sel_t")

            # seed0_t: ramp in scan units (env/adc); thr_r: same in lt units
            seed0_t = seedp.tile([P, 1], f32, tag="s0", name="seed0_t")
            nc.gpsimd.dma_start(seed0_t[:, :], seed_in[:, :])
            nc.gpsimd.dma_start(selw_sb[:, :], selw_in[:, :])
            nc.gpsimd.memset(cmt[:, :], cm)
            thr_r = seedp.tile([P, 1], f32, tag="tr", name="thr_r")
            nc.vector.tensor_scalar(thr_r[:, :], seed0_t[:, :], adc, None,
                                    op0=Alu.mult)

            # ---------- solve 0 ----------
            prep = {}

            def emit_prep(j):
                a = j * TF
                lts = ltp[:, a:a + TF]
                nc.sync.dma_start(lts, x_in[:, a:a + TF])
                # decimated EMA proxy (stride DEC), fp32 state
                pd_t = pdp.tile([P, TFD], f16, tag="pd", name="pd_t")
                nc.gpsimd.tensor_scalar(pd_t[:, :],
                                        ltp[:, a + DEC - 1:a + TF:DEC],
                                        gcm, None, op0=Alu.mult)
                pg_t = pgp.tile([P, TFD], f32, tag="pg", name="pg_t")
                pg_init = (prep[j - 1]["pg"][:, TFD - 1:TFD] if j > 0
                           else thr_r[:, 0:1])
                nc.vector.tensor_tensor_scan(pg_t[:, :], cmt[:, :],
                                             pd_t[:, :], pg_init,
                                             op0=Alu.mult, op1=Alu.add)
                # t0' = (lt > thr) + kr/adc, thr held per half-tile
                thr_lo = (prep[j - 1]["pg"][:, TFD - 1:TFD] if j > 0
                          else thr_r[:, 0:1])
                thr_hi = pg_t[:, TFD // 2 - 1:TFD // 2]
                t0_t = ttp.tile([P, TF], f16, tag="t0", name="t0_t")
                nc.vector.tensor_scalar(t0_t[:, 0:THALF], lts[:, 0:THALF],
                                        thr_lo, bk, op0=Alu.is_gt,
                                        op1=Alu.add)
                nc.vector.tensor_scalar(t0_t[:, THALF:TF], lts[:, THALF:TF],
                                        thr_hi, bk, op0=Alu.is_gt,
                                        op1=Alu.add)
                # c0 = 1 - adc*t0'
                c_t = cp.tile([P, TF], f32, tag="c", name="c_t")
                nc.scalar.activation(c_t[:, :], t0_t[:, :], Act.Copy,
                                     bias=1.0, scale=-adc)
                # d10' = t0' * lt
                d1_t = dp.tile([P, TF], f16, tag="d1", name="d1_t")
                nc.gpsimd.tensor_tensor(d1_t[:, :], t0_t[:, :], lts,
                                        op=Alu.mult)
                prep[j] = {"pg": pg_t, "c": c_t, "d1": d1_t}
                prep.pop(j - 2, None)

            envs = {}

            def emit_scan(j):
                a = j * TF
                env_t = efp.tile([P, TF], f32 if j == NT - 1 else f16,
                                 tag="ef32" if j == NT - 1 else "ef",
                                 name="env_t")
                init_ap = (envs[j - 1][:, TF - 1:TF] if j > 0
                           else seed0_t[:, 0:1])
                nc.vector.tensor_tensor_scan(env_t[:, :], prep[j]["c"][:, :],
                                             prep[j]["d1"][:, :], init_ap,
                                             op0=Alu.mult, op1=Alu.add)
                # gather solve-1 thresholds: thr1[j*NB1+k] = adc*env0'[249+250k]
                nc.gpsimd.tensor_scalar(
                    thrp[:, j * NB1:(j + 1) * NB1],
                    env_t[:, BL1 - 1:TF:BL1], adc, None, op0=Alu.mult)
                envs[j] = env_t
                envs.pop(j - 2, None)

            env0a = pers.tile([P, TF], f16, tag="e0a", name="env0a")

            emit_prep(0)
            for j in range(1, NT):
                emit_prep(j)
                emit_scan(j - 1)
                if j == 1:
                    # keep tile 0's env0' in lt units for the exact dec1
                    nc.vector.tensor_scalar(env0a[:, :], envs[0][:, :], adc,
                                            None, op0=Alu.mult)
            emit_scan(NT - 1)
            env_last = envs[NT - 1]

            # ---------- solve-1 prep ----------
            prep1 = {}

            def emit_prep1(j):
                a = j * TF
                lts = ltp[:, a:a + TF]
                t1_t = ttp.tile([P, TF], f16, tag="t0", name="t1_t")
                if j == 0:
                    # exact per-element decisions for the rising-envelope tile
                    nc.vector.tensor_tensor(t1_t[:, 1:], lts[:, 1:],
                                            env0a[:, :TF - 1], op=Alu.is_gt)
                    nc.vector.tensor_tensor(t1_t[:, 0:1], lts[:, 0:1],
                                            thr_r[:, 0:1], op=Alu.is_gt)
                    nc.vector.tensor_scalar(t1_t[:, :], t1_t[:, :], bk, None,
                                            op0=Alu.add)
                else:
                    for k in range(NB1):
                        col = j * NB1 + k - 1
                        thr = thrp[:, col:col + 1]
                        nc.gpsimd.tensor_scalar(
                            t1_t[:, k * BL1:(k + 1) * BL1],
                            lts[:, k * BL1:(k + 1) * BL1],
                            thr, bk, op0=Alu.is_gt, op1=Alu.add)
                c_t = cp.tile([P, TF], f32, tag="c", name="c1_t")
                nc.scalar.activation(c_t[:, :], t1_t[:, :], Act.Copy,
                                     bias=1.0, scale=-adc)
                d1_t = dp.tile([P, TF], f16, tag="d1", name="d11_t")
                d_eng = nc.vector if j < D11_DVE else nc.gpsimd
                d_eng.tensor_tensor(d1_t[:, :], t1_t[:, :], lts, op=Alu.mult)
                prep1[j] = {"c": c_t, "d1": d1_t}

            # emitted BEFORE the exchange: independent of the collective, so
            # the engines chew on them while the collective runs
            for j in range(min(LOOK, NT)):
                emit_prep1(j)

            # ---------- boundary exchange ----------
            bcol = bcolp.tile([P, 1], f32, tag="bcol", name="bcol")
            nc.vector.tensor_copy(bcol[:, :], env_last[:, TF - 1:TF])
            st1 = nc.sync.dma_start(bnd_loc[0:64], bcol[64:128, 0:1])
            st2 = nc.sync.dma_start(bnd_loc[64:128], bcol[0:64, 0:1])
            cc = nc.gpsimd.collective_compute(
                "AllGather", mybir.AluOpType.bypass,
                replica_groups=groups,
                ins=[bnd_loc[:]], outs=[bnd_all[:, :]],
            )
            add_dep_helper(cc.ins, st1.ins, sync=True,
                           reason="collective after bnd stores")
            add_dep_helper(cc.ins, st2.ins, sync=True,
                           reason="collective after bnd stores")
            for g in range(NCORES):
                ld = nc.sync.dma_start(bnd_sb[:, g:g + 1], bnd_all[g, :])
                add_dep_helper(ld.ins, cc.ins, sync=True,
                               reason="bnd load after collective")
            nc.vector.tensor_tensor(sel_t[:, :], bnd_sb[:, :], selw_sb[:, :],
                                    op=Alu.mult)
            seedy_t = seedp.tile([P, 1], f32, tag="sy", name="seedy_t")
            nc.vector.tensor_reduce(seedy_t[:, :], sel_t[:, :],
                                    axis=mybir.AxisListType.X, op=Alu.add)

            # ---------- solve 1 (final, in env/adc units) ----------
            prev_y = None
            for j in range(NT):
                a = j * TF
                y_t = yhp.tile([P, TF], f16, tag="yh", name="y_t")
                init_ap = (prev_y[:, TF - 1:TF] if j > 0 else seedy_t[:, 0:1])
                nc.vector.tensor_tensor_scan(y_t[:, :], prep1[j]["c"][:, :],
                                             prep1[j]["d1"][:, :], init_ap,
                                             op0=Alu.mult, op1=Alu.add)
                nc.sync.dma_start(y_out[:, a:a + TF], y_t[:, :])
                prev_y = y_t
                prep1.pop(j, None)
                if j + LOOK < NT:
                    emit_prep1(j + LOOK)
    nc.finalize()
    return nc


def _in_maps(x, ca, cr):
    x = np.asarray(x, dtype=np.float32)
    lt = np.abs(x) * np.float32(ALPHA)
    adc = np.float64(np.float32(cr) - np.float32(ca))
    maps = []
    t0 = np.empty(P, np.float64)
    for c in range(NCORES):
        t0[:64] = c * KCORE
        t0[64:] = c * KCORE + HALF
        # ramp seed in scan units (env/adc scale)
        seed0 = ((ALPHA / adc) * EQ *
                 (1.0 - np.exp(-t0 / TAU))).astype(np.float32)[:, None]
        selw = np.zeros((P, NCORES), np.float32)
        if c > 0:
            selw[:64, c - 1] = 1.0
        selw[64:, c] = 1.0
        s = c * KCORE
        xc = np.concatenate([lt[:, s:s + HALF], lt[:, s + HALF:s + KCORE]],
                            axis=0).astype(np.float16)
        maps.append({
            "xc": np.ascontiguousarray(xc),
            "seed0": seed0,
            "selw": selw,
        })
    return maps


def kernel(x, raw_attack, raw_release, sample_rate):
    from concourse.bass_utils import run_bass_kernel_spmd

    ca, cr = _coeffs(raw_attack, raw_release, sample_rate)
    key = (round(ca, 12), round(cr, 12), TF, GAMMA, CM_SAMPLES, DEC)
    if key not in _cache:
        _cache[key] = _build(ca, cr)
    nc = _cache[key]

    maps = _in_maps(x, ca, cr)
    res = run_bass_kernel_spmd(nc, maps, list(range(NCORES)), **_RUN_KWARGS)
    kernel.last_results = res

    adc = np.float32(np.float32(cr) - np.float32(ca))
    out_scale = np.float32(adc / np.float32(ALPHA))
    y = np.empty((B, L), np.float32)
    for c in range(NCORES):
        yc = np.asarray(res.results[c]["yc"], dtype=np.float32) * out_scale
        s = c * KCORE
        y[:, s:s + HALF] = yc[:64]
        y[:, s + HALF:s + KCORE] = yc[64:]
    return y


# revision 41
# speedup vs baseline: 1.6194x; 1.1202x over previous
"""Differentiable envelope follower on 8 Trainium2 NeuronCores.

Algorithm: the per-sample recurrence
    env[t] = c[t]*env[t-1] + (1-c[t])*|x[t]|,  c[t] = ca if |x[t]| > env[t-1] else cr
is solved by policy iteration: derive attack/release decisions elementwise
from a trajectory guess, solve the resulting LINEAR recurrence exactly with
tensor_tensor_scan (DVE-only), repeat.  All decisions are THRESHOLD compares
(tensor_scalar is_gt with a [P,1] ptr -- legal on Pool AND 4x-fast on DVE):
  solve 0: thr = decimated-EMA proxy of |x|, held per 1000-sample half-tile
  solve 1: thr = env0 itself, held per 250-sample block (plus an exact
           per-element compare for the first tile, where the envelope rises
           from zero and relative errors are magnified)
Max rel err 3.8e-3 vs the 2e-2 gate (validated in numpy at full scale
and in the simulator).

Cost structure (180.6us baseline -> 111.6us):
  - |x| and the f32->f16 downcast happen on the HOST; lt = f16(16*|x|).
  - both scans run in y' = env/adc units (adc = cr-ca) so the kk-affine
    collapses into the decision op:  t' = (lt > thr) + kr/adc  is ONE 4x
    tensor_scalar, d' = t'*lt one tensor_tensor, c = 1 - adc*t' one Act
    Copy.  Host multiplies the output by adc/16.
  - scans must run on DVE (ISA); d-products run on Pool, c-affines on Act,
    decision-TS ops split DVE/Pool -- all four engines near-balanced.

Sharding: L=480000 split across 8 cores (60000 each); 64 batch rows x 2
L-halves fill the 128 SBUF partitions ([128, 30000] per core).  Chunk-
boundary states are exchanged between solves with a tiny AllGather.
"""

import math
import numpy as np

# ---- problem constants (hardcoded per contract) ----
B = 64
L = 480000
NCORES = 8
KCORE = L // NCORES          # 60000 per core
HALF = KCORE // 2            # 30000 per partition-row
P = 128

# ---- tunables ----
ALPHA = 16.0                 # lt = ALPHA*|x| stored f16
TF = 2000                    # free-dim tile size (must divide HALF)
THALF = TF // 2              # solve-0 threshold hold (samples)
DEC = 8                      # proxy decimation stride
TFD = TF // DEC              # decimated proxy tile length (250)
BL1 = 250                    # solve-1 threshold hold (samples)
NB1 = TF // BL1              # solve-1 blocks per tile (8)
CM_SAMPLES = 4500.0          # EMA proxy time constant (samples)
GAMMA = 1.8                  # proxy -> envelope threshold scale
EQ = 1.42                    # equilibrium level of the init-guess ramp
TAU = 5600.0                 # time constant of the init-guess ramp
LOOK = 2                     # solve-1 tiles emitted before the exchange
D11_DVE = 2                  # tiles whose d11 TT runs on DVE (rest Pool)

_RUN_KWARGS = {}             # test.py can set {"trace": True}
_cache = {}


def _coeffs(raw_attack, raw_release, sample_rate):
    # Mirror reference._coefficients exactly (same jax ops, on CPU).
    import jax
    import jax.numpy as jnp

    with jax.default_device(jax.devices("cpu")[0]):
        attack_ms = 0.1 + jax.nn.sigmoid(jnp.asarray(np.float32(raw_attack))) * 49.9
        release_ms = 10.0 + jax.nn.sigmoid(jnp.asarray(np.float32(raw_release))) * 490.0
        attack_samples = attack_ms * float(sample_rate) / 1000.0
        release_samples = release_ms * float(sample_rate) / 1000.0
        ca = jnp.exp(-1.0 / attack_samples)
        cr = jnp.exp(-1.0 / release_samples)
        return float(ca), float(cr)


def _build(ca, cr):
    import concourse.bass as bass
    import concourse.bacc as bacc
    import concourse.tile as tile
    from concourse import mybir
    from concourse.tile_rust import add_dep_helper
    from contextlib import ExitStack

    f32 = mybir.dt.float32
    f16 = mybir.dt.float16
    Alu = mybir.AluOpType
    Act = mybir.ActivationFunctionType
    NT = HALF // TF
    assert NT * TF == HALF

    adc = float(np.float32(cr) - np.float32(ca))         # > 0
    kr = float(np.float32(1.0) - np.float32(cr))
    bk = float(np.float32(kr) / np.float32(adc))         # t' offset (~0.109)
    cm = float(np.float32(math.exp(-DEC / CM_SAMPLES)))  # per decimated step
    gcm = float(np.float32(GAMMA) * (np.float32(1.0) - np.float32(cm)))

    nc = bacc.Bacc("TRN2", target_bir_lowering=False, debug=False,
                   num_devices=NCORES)

    x_in = nc.dram_tensor("xc", [P, HALF], f16, kind="ExternalInput")
    seed_in = nc.dram_tensor("seed0", [P, 1], f32, kind="ExternalInput")
    selw_in = nc.dram_tensor("selw", [P, NCORES], f32, kind="ExternalInput")
    y_out = nc.dram_tensor("yc", [P, HALF], f16, kind="ExternalOutput")
    bnd_loc = nc.dram_tensor("bnd_loc", [P], f32)
    bnd_all = nc.dram_tensor("bnd_all", [NCORES, P], f32, addr_space="Shared")

    nc.alloc_semaphore("bnd_dma")
    nc.alloc_semaphore("bnd_cc")
    groups = [list(range(NCORES))]

    with tile.TileContext(nc) as tc:
        with ExitStack() as ctx:
            pers = ctx.enter_context(tc.tile_pool(name="pers", bufs=1))
            pgp = ctx.enter_context(tc.tile_pool(name="pg", bufs=2))
            pdp = ctx.enter_context(tc.tile_pool(name="pd", bufs=2))
            ttp = ctx.enter_context(tc.tile_pool(name="tt", bufs=2))
            cp = ctx.enter_context(tc.tile_pool(name="c", bufs=3))
            dp = ctx.enter_context(tc.tile_pool(name="d1", bufs=4))
            efp = ctx.enter_context(tc.tile_pool(name="ef", bufs=2))
            yhp = ctx.enter_context(tc.tile_pool(name="yh", bufs=2))
            xtr = ctx.enter_context(tc.tile_pool(name="xtr", bufs=2))
            bcolp = ctx.enter_context(tc.tile_pool(name="bcol", bufs=2))
            seedp = ctx.enter_context(tc.tile_pool(name="seed", bufs=2))

            ltp = pers.tile([P, HALF], f16, tag="lt", name="ltp")
            t1p = pers.tile([P, HALF], f16, tag="t1", name="t1p")
            thrp = pers.tile([P, NT * NB1], f32, tag="thr", name="thrp")
            cmt = pers.tile([P, TFD], f32, tag="cmt", name="cmt")
            selw_sb = pers.tile([P, NCORES], f32, tag="selw", name="selw_sb")
            bnd_sb = pers.tile([P, NCORES], f32, tag="bnd", name="bnd_sb")
            sel_t = pers.tile([P, NCORES], f32, tag="sel", name="sel_t")

            # seed0_t: ramp in scan units (env/adc); thr_r: same in lt units
            seed0_t = seedp.tile([P, 1], f32, tag="s0", name="seed0_t")
            nc.gpsimd.dma_start(seed0_t[:, :], seed_in[:, :])
            nc.gpsimd.dma_start(selw_sb[:, :], selw_in[:, :])
            nc.gpsimd.memset(cmt[:, :], cm)
            thr_r = seedp.tile([P, 1], f32, tag="tr", name="thr_r")
            nc.vector.tensor_scalar(thr_r[:, :], seed0_t[:, :], adc, None,
                                    op0=Alu.mult)

            # ---------- solve 0 ----------
            prep = {}

            def emit_prep0():
                # tile 0 in two 1000-col halves so the first env0 scan can
                # start after ~2us instead of ~6 (same fp32 chain; the halves
                # write slices of full-size tiles to avoid new SBUF tags)
                t0_t = ttp.tile([P, TF], f16, tag="t0", name="t0z_t")
                c_t = cp.tile([P, TF], f32, tag="c", name="c0z_t")
                d1_t = dp.tile([P, TF], f16, tag="d1", name="d0z_t")
                pgs = []
                for s in range(2):
                    a = s * THALF
                    lts = ltp[:, a:a + THALF]
                    nc.sync.dma_start(lts, x_in[:, a:a + THALF])
                    pd_t = pdp.tile([P, TFD // 2], f16, tag="pd0", name="pd0_t")
                    nc.scalar.activation(pd_t[:, :],
                                         ltp[:, a + DEC - 1:a + THALF:DEC],
                                         Act.Copy, bias=0.0, scale=gcm)
                    pg_t = pgp.tile([P, TFD // 2], f32, tag="pg0", name="pg0_t")
                    pg_init = pgs[0][:, TFD // 2 - 1:TFD // 2] if s else thr_r[:, 0:1]
                    nc.vector.tensor_tensor_scan(pg_t[:, :],
                                                 cmt[:, :TFD // 2],
                                                 pd_t[:, :], pg_init,
                                                 op0=Alu.mult, op1=Alu.add)
                    pgs.append(pg_t)
                    thr = pgs[0][:, TFD // 2 - 1:TFD // 2] if s else thr_r[:, 0:1]
                    nc.vector.tensor_scalar(t0_t[:, a:a + THALF], lts, thr,
                                            bk, op0=Alu.is_gt, op1=Alu.add)
                    nc.scalar.activation(c_t[:, a:a + THALF],
                                         t0_t[:, a:a + THALF], Act.Copy,
                                         bias=1.0, scale=-adc)
                    nc.gpsimd.tensor_tensor(d1_t[:, a:a + THALF],
                                            t0_t[:, a:a + THALF], lts,
                                            op=Alu.mult)
                prep[0] = {"pg": pgs[1], "c": c_t, "d1": d1_t,
                           "pg_off": TFD // 2, "split": True}

            def emit_prep(j):
                a = j * TF
                lts = ltp[:, a:a + TF]
                nc.sync.dma_start(lts, x_in[:, a:a + TF])
                # decimated EMA proxy (stride DEC), fp32 state
                pd_t = pdp.tile([P, TFD], f16, tag="pd", name="pd_t")
                nc.scalar.activation(pd_t[:, :],
                                     ltp[:, a + DEC - 1:a + TF:DEC],
                                     Act.Copy, bias=0.0, scale=gcm)
                pg_t = pgp.tile([P, TFD], f32, tag="pg", name="pg_t")
                pprev = prep[j - 1]
                pg_init = pprev["pg"][:, pprev.get("pg_off", TFD) - 1:
                                      pprev.get("pg_off", TFD)]
                nc.vector.tensor_tensor_scan(pg_t[:, :], cmt[:, :],
                                             pd_t[:, :], pg_init,
                                             op0=Alu.mult, op1=Alu.add)
                # t0' = (lt > thr) + kr/adc, thr held per half-tile
                po = pprev.get("pg_off", TFD)
                thr_lo = pprev["pg"][:, po - 1:po]
                thr_hi = pg_t[:, TFD // 2 - 1:TFD // 2]
                t0_t = ttp.tile([P, TF], f16, tag="t0", name="t0_t")
                nc.vector.tensor_scalar(t0_t[:, 0:THALF], lts[:, 0:THALF],
                                        thr_lo, bk, op0=Alu.is_gt,
                                        op1=Alu.add)
                nc.vector.tensor_scalar(t0_t[:, THALF:TF], lts[:, THALF:TF],
                                        thr_hi, bk, op0=Alu.is_gt,
                                        op1=Alu.add)
                # c0 = 1 - adc*t0'
                c_t = cp.tile([P, TF], f32, tag="c", name="c_t")
                nc.scalar.activation(c_t[:, :], t0_t[:, :], Act.Copy,
                                     bias=1.0, scale=-adc)
                # d10' = t0' * lt
                d1_t = dp.tile([P, TF], f16, tag="d1", name="d1_t")
                nc.gpsimd.tensor_tensor(d1_t[:, :], t0_t[:, :], lts,
                                        op=Alu.mult)
                prep[j] = {"pg": pg_t, "c": c_t, "d1": d1_t}
                prep.pop(j - 2, None)

            envs = {}

            def emit_scan(j):
                a = j * TF
                env_t = efp.tile([P, TF], f32 if j == NT - 1 else f16,
                                 tag="ef32" if j == NT - 1 else "ef",
                                 name="env_t",
                                 bufs=1 if j == NT - 1 else None)
                if prep[j].get("split"):
                    nc.vector.tensor_tensor_scan(
                        env_t[:, 0:THALF], prep[j]["c"][:, 0:THALF],
                        prep[j]["d1"][:, 0:THALF], seed0_t[:, 0:1],
                        op0=Alu.mult, op1=Alu.add)
                    nc.vector.tensor_tensor_scan(
                        env_t[:, THALF:TF], prep[j]["c"][:, THALF:TF],
                        prep[j]["d1"][:, THALF:TF], env_t[:, THALF - 1:THALF],
                        op0=Alu.mult, op1=Alu.add)
                else:
                    init_ap = (envs[j - 1][:, TF - 1:TF] if j > 0
                               else seed0_t[:, 0:1])
                    nc.vector.tensor_tensor_scan(env_t[:, :],
                                                 prep[j]["c"][:, :],
                                                 prep[j]["d1"][:, :], init_ap,
                                                 op0=Alu.mult, op1=Alu.add)
                # gather solve-1 thresholds: thr1[j*NB1+k] = adc*env0'[249+250k]
                nc.gpsimd.tensor_scalar(
                    thrp[:, j * NB1:(j + 1) * NB1],
                    env_t[:, BL1 - 1:TF:BL1], adc, None, op0=Alu.mult)
                envs[j] = env_t
                envs.pop(j - 2, None)

            env0a = pers.tile([P, TF], f16, tag="e0a", name="env0a")

            def emit_t1(j):
                # solve-1 decision plane, filled DURING solve 0 so the Pool
                # engine's solve-0 slack absorbs it (the y-phase then only
                # carries d11)
                a = j * TF
                lts = ltp[:, a:a + TF]
                if j == 0:
                    # exact per-element decisions for the rising-envelope tile
                    dx_t = ttp.tile([P, TF], f16, tag="t0", name="dx_t")
                    nc.vector.tensor_tensor(dx_t[:, 1:], lts[:, 1:],
                                            env0a[:, :TF - 1], op=Alu.is_gt)
                    nc.vector.tensor_tensor(dx_t[:, 0:1], lts[:, 0:1],
                                            thr_r[:, 0:1], op=Alu.is_gt)
                    nc.vector.tensor_scalar(t1p[:, a:a + TF], dx_t[:, :],
                                            bk, None, op0=Alu.add)
                else:
                    for k in range(NB1):
                        col = j * NB1 + k - 1
                        thr = thrp[:, col:col + 1]
                        eng = nc.vector if k <= 1 else nc.gpsimd
                        eng.tensor_scalar(
                            t1p[:, a + k * BL1:a + (k + 1) * BL1],
                            lts[:, k * BL1:(k + 1) * BL1],
                            thr, bk, op0=Alu.is_gt, op1=Alu.add)

            emit_prep0()
            for j in range(1, NT):
                emit_prep(j)
                emit_scan(j - 1)
                if j == 1:
                    # keep tile 0's env0' in lt units for the exact dec1
                    nc.vector.tensor_scalar(env0a[:, :], envs[0][:, :], adc,
                                            None, op0=Alu.mult)
                    emit_t1(0)
                else:
                    emit_t1(j - 1)
            emit_scan(NT - 1)
            env_last = envs[NT - 1]
            # ---------- boundary exchange (emitted before the last
            # tile's t1 blocks so the collective issues immediately) ----
            st1 = nc.sync.dma_start(bnd_loc[0:64],
                                    env_last[64:128, TF - 1:TF])
            st2 = nc.scalar.dma_start(bnd_loc[64:128],
                                      env_last[0:64, TF - 1:TF])
            cc = nc.gpsimd.collective_compute(
                "AllGather", mybir.AluOpType.bypass,
                replica_groups=groups,
                ins=[bnd_loc[:]], outs=[bnd_all[:, :]],
            )
            add_dep_helper(cc.ins, st1.ins, sync=True,
                           reason="collective after bnd stores")
            add_dep_helper(cc.ins, st2.ins, sync=True,
                           reason="collective after bnd stores")
            ld = nc.sync.dma_start(bnd_sb[:, :], bnd_all[:, :].transpose([1, 0]))
            add_dep_helper(ld.ins, cc.ins, sync=True,
                           reason="bnd load after collective")
            seedy_t = seedp.tile([P, 1], f32, tag="sy", name="seedy_t")
            nc.vector.scalar_tensor_tensor(sel_t[:, :], bnd_sb[:, :], 1.0,
                                           selw_sb[:, :], op0=Alu.mult,
                                           op1=Alu.mult,
                                           accum_out=seedy_t[:, 0:1])

            emit_t1(NT - 1)

            # ---------- solve-1 prep (c1 + d11 from the t1 plane) ----------
            prep1 = {}

            def emit_prep1(j):
                a = j * TF
                lts = ltp[:, a:a + TF]
                t1s = t1p[:, a:a + TF]
                c_t = cp.tile([P, TF], f32, tag="c", name="c1_t")
                nc.scalar.activation(c_t[:, :], t1s, Act.Copy,
                                     bias=1.0, scale=-adc)
                d1_t = dp.tile([P, TF], f16, tag="d1", name="d11_t")
                d_eng = nc.vector if j < D11_DVE else nc.gpsimd
                d_eng.tensor_tensor(d1_t[:, :], t1s, lts, op=Alu.mult)
                prep1[j] = {"c": c_t, "d1": d1_t}

            # emitted BEFORE the exchange: independent of the collective, so
            # the engines chew on them while the collective runs
            for j in range(min(LOOK, NT)):
                emit_prep1(j)

            # ---------- solve 1 (final, in env/adc units) ----------
            prev_y = None
            for j in range(NT):
                a = j * TF
                y_t = yhp.tile([P, TF], f16, tag="yh", name="y_t")
                init_ap = (prev_y[:, TF - 1:TF] if j > 0 else seedy_t[:, 0:1])
                if j == NT - 1:
                    h = TF // 2
                    nc.vector.tensor_tensor_scan(
                        y_t[:, 0:h], prep1[j]["c"][:, 0:h],
                        prep1[j]["d1"][:, 0:h], init_ap,
                        op0=Alu.mult, op1=Alu.add)
                    nc.sync.dma_start(y_out[:, a:a + h], y_t[:, 0:h])
                    nc.vector.tensor_tensor_scan(
                        y_t[:, h:TF], prep1[j]["c"][:, h:TF],
                        prep1[j]["d1"][:, h:TF], y_t[:, h - 1:h],
                        op0=Alu.mult, op1=Alu.add)
                    nc.scalar.dma_start(y_out[:, a + h:a + TF], y_t[:, h:TF])
                else:
                    nc.vector.tensor_tensor_scan(
                        y_t[:, :], prep1[j]["c"][:, :],
                        prep1[j]["d1"][:, :], init_ap,
                        op0=Alu.mult, op1=Alu.add)
                    nc.sync.dma_start(y_out[:, a:a + TF], y_t[:, :])
                prev_y = y_t
                prep1.pop(j, None)
                if j + LOOK < NT:
                    emit_prep1(j + LOOK)
    nc.finalize()
    return nc


def _in_maps(x, ca, cr):
    x = np.asarray(x, dtype=np.float32)
    lt = np.abs(x) * np.float32(ALPHA)
    adc = np.float64(np.float32(cr) - np.float32(ca))
    maps = []
    t0 = np.empty(P, np.float64)
    for c in range(NCORES):
        t0[:64] = c * KCORE
        t0[64:] = c * KCORE + HALF
        # ramp seed in scan units (env/adc scale)
        seed0 = ((ALPHA / adc) * EQ *
                 (1.0 - np.exp(-t0 / TAU))).astype(np.float32)[:, None]
        selw = np.zeros((P, NCORES), np.float32)
        if c > 0:
            selw[:64, c - 1] = 1.0
        selw[64:, c] = 1.0
        s = c * KCORE
        xc = np.concatenate([lt[:, s:s + HALF], lt[:, s + HALF:s + KCORE]],
                            axis=0).astype(np.float16)
        maps.append({
            "xc": np.ascontiguousarray(xc),
            "seed0": seed0,
            "selw": selw,
        })
    return maps


def kernel(x, raw_attack, raw_release, sample_rate):
    from concourse.bass_utils import run_bass_kernel_spmd

    ca, cr = _coeffs(raw_attack, raw_release, sample_rate)
    key = (round(ca, 12), round(cr, 12), TF, GAMMA, CM_SAMPLES, DEC)
    if key not in _cache:
        _cache[key] = _build(ca, cr)
    nc = _cache[key]

    maps = _in_maps(x, ca, cr)
    res = run_bass_kernel_spmd(nc, maps, list(range(NCORES)), **_RUN_KWARGS)
    kernel.last_results = res

    adc = np.float32(np.float32(cr) - np.float32(ca))
    out_scale = np.float32(adc / np.float32(ALPHA))
    y = np.empty((B, L), np.float32)
    for c in range(NCORES):
        yc = np.asarray(res.results[c]["yc"], dtype=np.float32) * out_scale
        s = c * KCORE
        y[:, s:s + HALF] = yc[:64]
        y[:, s + HALF:s + KCORE] = yc[64:]
    return y
